# revision 1
# baseline (speedup 1.0000x reference)
"""Trainium2 Bass kernel: 1-D horizontal cost volume (9 disparities).

out[b, j, h, w] = mean_c( f1[b,c,h,w] * zeropad_w(f2)[b,c,h,w+j] ),  j = 0..8.

Sharding: 8 cores, each handles one (batch, H-half) slice [C=128, 96, W=640].
No halo needed (shift is along W only).

Per-core algorithm, per image row r (W split into 5 tiles of 128):
  1. Band matmul on TensorE: psum[m, n] = sum_c f1[c, w0+m]/C * f2pad[c, w0+n-4]
     (lhsT = f1 cast to bf16 and pre-scaled by 1/C on ScalarE, rhs = f2 bf16 with
     4-column zero pads). All 9 disparities live on diagonals psum[m, m+j].
  2. SBUF-side DMA access patterns cannot step partition+byte diagonally (the
     HW DGE wraps the per-partition byte offset mod 16B), so the diagonals are
     extracted through a DRAM scratch roundtrip.  To keep the parked bytes
     small, only the 40-column window [32g, 32g+40) of each 32-partition group
     g is parked (it covers every diagonal element of that group).  Group g's
     park writes window column c of chunk u from partition m = 32g+b to DRAM
     address  1601*32g + 1600*b + 40*u + c ;  the diagonal element [m, m+j]
     (window column c = b+j) then sits at  1601*m + 40*u + j  -- affine in
     (m, u, j), so one 3-dim DMA per block reads every diagonal back:
     D[m, 9u+j] = X[1601*m + 40*u + j].   (u indexes rows x 5 w-tiles; (row,
     tile) folds into one AP dim because a chunk is exactly 136 elements.)
  3. PE-transpose D [128, 45] -> [45, 128] per row so j becomes the partition
     dim, copy to SBUF fp32, one 3D DMA per w-tile per 16 rows writes
     out[j, r, 128t+w].

Queue layout (to avoid head-of-line blocking on in-order queues):
  SP (sync): every DMA -- next block's input loads (prefetched one block
      ahead so casts never wait), previous block's diagonal readback, this
      block's park x4, and the per-16-row-group output writes.
  Act (scalar): f1 scale-cast, half the PSUM->SBUF copies, half the oblk
      copies.
  DVE (vector): f2 cast, the other halves.
  PE: band matmuls, then two-blocks-ago transposes.
The readback of block N is emitted in block N+1, and the tail of block N
(transposes/oblk copies/out writes) at the END of block N+2 (after the
matmuls, so the in-order PE queue never holds matmuls hostage to an
in-flight readback).  The last two blocks are 4 rows instead of 8 to
shorten the pipeline drain after the final input load.
"""

import numpy as np

import concourse.bass as bass
import concourse.bacc as bacc
import concourse.tile as tile
from concourse import mybir
from concourse import bass_utils
from concourse.masks import make_identity

B, C, H, W = 4, 128, 192, 640
NJ = 9               # 2*4+1 disparities
NCORES = 8
HS = B * H // NCORES  # 96 rows per core
NT = W // 128         # 5 w-tiles per row
ROWBLK = 8            # max rows per DMA/pipeline block

F32 = mybir.dt.float32
BF16 = mybir.dt.bfloat16
PARK_DT = BF16        # dtype of the parked band-matmul results

_CACHE: dict = {}
TRACE = False  # set True (e.g. from test.py) to capture an NTFF profile
LAST_RESULT = None  # BassKernelResults of the most recent run when TRACE


def _block_plan(hs: int):
    """Block sizes: all 8 rows except the last two (4 rows each).  Output
    writes are batched per group; the trailing groups are small so the final
    out writes wait on as little of the pipeline drain as possible."""
    sizes = [ROWBLK] * (hs // ROWBLK - 1) + [ROWBLK // 2, ROWBLK // 2]
    groups = [16, 16, 16, 16, 16, 16]
    assert sum(sizes) == hs and sum(groups) == hs
    blocks = []  # (r0, nr, group_r0, group_off, group_rows, last_in_group)
    r0 = 0
    gi = 0
    group_r0 = 0
    group_fill = 0
    for nr in sizes:
        group_off = group_fill
        group_fill += nr
        last = group_fill == groups[gi]
        blocks.append((r0, nr, group_r0, group_off, groups[gi], last))
        r0 += nr
        if last:
            group_r0 = r0
            group_fill = 0
            gi += 1
    assert group_fill == 0
    return blocks


def _build_program(hs: int = HS):
    from contextlib import ExitStack

    plan = _block_plan(hs)
    nblk = len(plan)
    nc = bacc.Bacc("TRN2", target_bir_lowering=False, debug=False)
    f1 = nc.dram_tensor("f1", [C, hs, W], F32, kind="ExternalInput")
    f2 = nc.dram_tensor("f2", [C, hs, W], F32, kind="ExternalInput")
    out = nc.dram_tensor("out", [NJ, hs, W], F32, kind="ExternalOutput")

    with tile.TileContext(nc) as tc, ExitStack() as ctx:
        consts = ctx.enter_context(tc.tile_pool(name="consts", bufs=1))
        loads = ctx.enter_context(tc.tile_pool(name="loads", bufs=3))
        casts = ctx.enter_context(tc.tile_pool(name="casts", bufs=2))
        srow = ctx.enter_context(tc.tile_pool(name="srow", bufs=2))
        dstk = ctx.enter_context(tc.tile_pool(name="dstk", bufs=3))
        outp = ctx.enter_context(tc.tile_pool(name="outp", bufs=2))
        ppool = ctx.enter_context(tc.tile_pool(name="psum1", bufs=3, space="PSUM"))
        ppool2 = ctx.enter_context(tc.tile_pool(name="psum2", bufs=2, space="PSUM"))
        xpool = ctx.enter_context(tc.tile_pool(name="xpark", bufs=3, space="DRAM"))

        ident = consts.tile([128, 128], BF16)
        make_identity(nc, ident)

        # f2b double buffer with the 4+4 zero pad columns written once.
        f2bufs = [
            consts.tile([128, ROWBLK, W + 8], BF16, name=f"f2buf{i}")
            for i in range(2)
        ]
        for fb in f2bufs:
            nc.gpsimd.memset(fb[:, :, 0:4], 0.0)
            nc.gpsimd.memset(fb[:, :, W + 4 : W + 8], 0.0)

        # software-pipeline state: readback lags park by 1 block, the tail
        # (transpose/oblk/out) lags the readback by 1 more block.
        state: dict = {}

        def emit_readback(rb: int):
            """diagonal readback for block rb (parked during block rb)."""
            xblk, nu, pstride = state.pop(("xblk", rb))
            dstack = dstk.tile([128, nu * NJ], PARK_DT)
            src = bass.AP(
                xblk.tensor, xblk.offset, [[pstride, 128], [40, nu], [1, NJ]]
            )
            dst = bass.AP(
                dstack.tensor, dstack.offset, [[nu * NJ, 128], [NJ, nu], [1, NJ]]
            )
            nc.sync.dma_start(out=dst, in_=src)
            state[("dstack", rb)] = dstack

        def emit_tail(rb: int):
            """transposes + oblk copies for block rb; outs once per group."""
            dstack = state.pop(("dstack", rb))
            r0, nr, group_r0, group_off, group_rows, last = plan[rb]
            if group_off == 0:
                state["oblk2"] = outp.tile(
                    [NT * NJ, group_rows, 128], F32, name="oblk2"
                )
            oblk = state["oblk2"]
            for dr in range(nr):
                ps2 = ppool2.tile([NT * NJ, 128], PARK_DT)
                nc.tensor.transpose(ps2, dstack[:, 45 * dr : 45 * (dr + 1)], ident)
                if dr % 2 == 0:
                    nc.scalar.copy(oblk[:, group_off + dr, :], ps2)
                else:
                    nc.vector.tensor_copy(oblk[:, group_off + dr, :], ps2)
            if last:
                for t in range(NT):
                    osrc = bass.AP(
                        oblk.tensor,
                        oblk.offset + NJ * t * (group_rows * 128),
                        [[group_rows * 128, NJ], [128, group_rows], [1, 128]],
                    )
                    odst = bass.AP(
                        out.ap().tensor,
                        group_r0 * W + 128 * t,
                        [[hs * W, NJ], [W, group_rows], [1, 128]],
                    )
                    nc.sync.dma_start(out=odst, in_=osrc)

        def emit_loads(rb: int):
            r0, nr, _, _, _, _ = plan[rb]
            f1row = loads.tile([128, nr, W], F32, name="f1row")
            nc.sync.dma_start(out=f1row, in_=f1.ap()[:, r0 : r0 + nr, :])
            f2row = loads.tile([128, nr, W], F32, name="f2row")
            nc.sync.dma_start(out=f2row, in_=f2.ap()[:, r0 : r0 + nr, :])
            state[("rows", rb)] = (f1row, f2row)

        emit_loads(0)
        for rb, (r0, nr, _, _, _, _) in enumerate(plan):
            nu = nr * NT
            pstride = nu * 40 + 1
            # ---- prefetch the next block's rows (SP queue: shallow deps) ----
            if rb + 1 < nblk:
                emit_loads(rb + 1)

            # ---- cast to bf16 (f1 pre-scaled by 1/C); loads arrived during
            # the previous block, so these dispatch without waiting ----
            f1row, f2row = state.pop(("rows", rb))
            f1b = casts.tile([128, nr, W], BF16, name="f1b")
            nc.scalar.mul(f1b, f1row, 1.0 / C)
            f2b = f2bufs[rb % 2]
            nc.vector.tensor_copy(f2b[:, :nr, 4 : W + 4], f2row)

            # ---- previous block's diagonal readback (on SP, early) ----
            if rb > 0:
                emit_readback(rb - 1)

            # ---- band matmuls + PSUM -> SBUF copies ----
            sblk = srow.tile([128, nr, NT * 136], PARK_DT, name="sblk")
            for dr in range(nr):
                psA = ppool.tile([128, 3, 136], F32, tag="psA")
                psB = ppool.tile([128, 2, 136], F32, tag="psB")
                for t in range(NT):
                    ps = psA[:, t, :] if t < 3 else psB[:, t - 3, :]
                    nc.tensor.matmul(
                        ps,
                        f1b[:, dr, 128 * t : 128 * (t + 1)],
                        f2b[:, dr, 128 * t : 128 * t + 136],
                        start=True,
                        stop=True,
                    )
                if dr % 2 == 0:
                    nc.scalar.copy(sblk[:, dr, 0 : 3 * 136], psA)
                    nc.vector.tensor_copy(sblk[:, dr, 3 * 136 : 5 * 136], psB)
                else:
                    nc.vector.tensor_copy(sblk[:, dr, 0 : 3 * 136], psA)
                    nc.scalar.copy(sblk[:, dr, 3 * 136 : 5 * 136], psB)

            # ---- park the 40-col window of each 32-partition group ----
            # src: sblk[32g + b, chunk u, 32g + c]  ((row, tile) folds into u
            #      because a chunk is exactly 136 elements)
            # dst: X[1601*32g + 1600*b + 40*u + c]
            xblk = xpool.tile([128, pstride], PARK_DT, name="xblk")
            for g in range(4):
                psrc = bass.AP(
                    sblk.tensor,
                    sblk.offset + 32 * g * (nr * NT * 136) + 32 * g,
                    [[nr * NT * 136, 32], [136, nu], [1, 40]],
                )
                pdst = bass.AP(
                    xblk.tensor,
                    xblk.offset + 32 * g * pstride,
                    [[pstride - 1, 32], [40, nu], [1, 40]],
                )
                nc.sync.dma_start(out=pdst, in_=psrc)
            state[("xblk", rb)] = (xblk, nu, pstride)

            # ---- two-blocks-ago tail (after this block's matmuls so the PE
            # queue never holds them hostage to an in-flight readback) ----
            if rb > 1:
                emit_tail(rb - 2)

        emit_tail(nblk - 2)
        emit_readback(nblk - 1)
        emit_tail(nblk - 1)

    nc.compile()
    return nc


def _get_nc():
    if "nc" not in _CACHE:
        _CACHE["nc"] = _build_program()
    return _CACHE["nc"]


def kernel(feature1: np.ndarray, feature2: np.ndarray) -> np.ndarray:
    f1 = np.asarray(feature1, dtype=np.float32)
    f2 = np.asarray(feature2, dtype=np.float32)
    assert f1.shape == (B, C, H, W) and f2.shape == (B, C, H, W)

    nc = _get_nc()
    in_maps = []
    for core in range(NCORES):
        b = core // 2
        h0 = (core % 2) * HS
        in_maps.append(
            {
                "f1": np.ascontiguousarray(f1[b, :, h0 : h0 + HS, :]),
                "f2": np.ascontiguousarray(f2[b, :, h0 : h0 + HS, :]),
            }
        )
    try:
        res = bass_utils.run_bass_kernel_spmd(
            nc, in_maps, core_ids=list(range(NCORES)), trace=TRACE
        )
    except ModuleNotFoundError:
        if not TRACE:
            raise
        # NTFF profile hook unavailable (e.g. axon container): run untraced.
        res = bass_utils.run_bass_kernel_spmd(
            nc, in_maps, core_ids=list(range(NCORES)), trace=False
        )
    if TRACE:
        global LAST_RESULT
        LAST_RESULT = res
    outv = np.empty((B, NJ, H, W), dtype=np.float32)
    for core in range(NCORES):
        b = core // 2
        h0 = (core % 2) * HS
        outv[b, :, h0 : h0 + HS, :] = res.results[core]["out"]
    return outv

